# revision 6
# baseline (speedup 1.0000x reference)
"""Masked-LSTM scan (MemoryAgent) Trainium2 kernel.

Strategy: the done-mask (p=0.5 per step) zeroes the carried state, so each
batch row's 512-step timeline splits into independent episodes at reset
points. Episodes are bin-packed (host side, exact) into 2048 columns split
into two groups per core: group A columns of SA steps and group B columns of
SB steps (SA+SB=32), 128 columns of each group per core. The device kernel
interleaves one A-step and one B-step per macro-step — two independent
dependency chains that keep PE/ACT/DVE busy. Per step: gate matmuls (x-part
+ bias + recurrent part accumulated in PSUM, float32r), sigmoid/tanh on
ScalarE, c/h elementwise updates on VectorE, and the masked state transpose
for the next step fused into a TensorE matmul against a per-step diagonal
mask matrix.
"""

import sys

try:
    import concourse  # noqa: F401
except ImportError:  # pragma: no cover
    sys.path.insert(0, "/opt/trn_rl_repo")

import numpy as np

import concourse.bacc as bacc
import concourse.mybir as mybir
import concourse.tile as tile
from concourse import bass_utils

T, B, I, H = 512, 64, 512, 256
NCORES = 8
SLOTS = 128
NCOLS = NCORES * SLOTS          # columns per group
F32 = mybir.dt.float32
F32R = mybir.dt.float32r

# PyTorch gate order in the weights is [i, f, g, o]; we compute in order
# [f, i, o, g] so sigmoid covers one contiguous [0:768] slice and tanh [768:1024].
_GATE_PERM = np.concatenate([
    np.arange(256, 512),    # f
    np.arange(0, 256),      # i
    np.arange(768, 1024),   # o
    np.arange(512, 768),    # g
])


# ---------------------------------------------------------------- packing ---

def _episodes(done):
    heads, tails, mids = [], [], []
    for b in range(B):
        starts = np.flatnonzero(done[1:, b]) + 1
        starts = [0] + starts.tolist()
        ends = starts[1:] + [T]
        row = [(b, t0, e - t0) for t0, e in zip(starts, ends)]
        assert len(row) >= 2, "single-episode row unsupported"
        heads.append(row[0])
        tails.append(row[-1])
        mids.extend(row[1:-1])
    return heads, tails, mids


def _pack(done):
    """Bin-pack episodes into two groups of NCOLS columns: group A with SA
    steps, group B with SB steps, SA+SB = T*B/NCOLS.

    Heads (carrying h0/c0) start A-columns 0..63; tails (whose final c is
    cT[b]) end A-columns 64..127. Episodes longer than SA go to B columns.
    Returns (SA, SB, col_t, col_b, valid) with col_* of shape [2*NCOLS, Smax]
    (group A rows first), padded positions invalid.
    """
    heads, tails, mids = _episodes(done)
    stot = (T * B) // NCOLS                      # SA + SB
    maxmid = max(l for (_, _, l) in mids)
    maxht = max(l for (_, _, l) in heads + tails)
    SB = max(maxmid, (stot + 1) // 2)
    SA = stot - SB
    assert SA >= 1 and maxht <= SA, (SA, SB, maxht)

    nb = 2 * NCOLS
    cap = np.array([SA] * NCOLS + [SB] * NCOLS)
    head_of = [None] * nb
    tail_of = [None] * nb
    mids_of = [[] for _ in range(nb)]
    rem = cap.copy()
    for b, ep in enumerate(heads):
        head_of[b] = ep
        rem[b] -= ep[2]
    for b, ep in enumerate(tails):
        tail_of[64 + b] = ep
        rem[64 + b] -= ep[2]
    assert (rem >= 0).all()

    big = sorted([m for m in mids if m[2] > 1], key=lambda e: -e[2])
    unit = [m for m in mids if m[2] == 1]

    by_rem = [[] for _ in range(SB + 1)]
    for c in range(nb):
        by_rem[rem[c]].append(c)
    for ep in big:
        L = ep[2]
        for r in range(L, SB + 1):
            if by_rem[r]:
                c = by_rem[r].pop()
                mids_of[c].append(ep)
                rem[c] -= L
                by_rem[r - L].append(c)
                break
        else:
            raise AssertionError("packing failed for episode of length %d" % L)
    ui = 0
    for c in range(nb):
        while rem[c] > 0 and ui < len(unit):
            mids_of[c].append(unit[ui])
            ui += 1
            rem[c] -= 1
    assert ui == len(unit) and (rem == 0).all()

    Smax = SB
    col_t = np.zeros((nb, Smax), np.int32)
    col_b = np.zeros((nb, Smax), np.int32)
    valid = np.zeros((nb, Smax), bool)
    for c in range(nb):
        items = []
        if head_of[c]:
            items.append(head_of[c])
        items.extend(mids_of[c])
        if tail_of[c]:
            items.append(tail_of[c])
        s = 0
        for b, t0, L in items:
            col_t[c, s:s + L] = np.arange(t0, t0 + L)
            col_b[c, s:s + L] = b
            valid[c, s:s + L] = True
            s += L
        assert s == cap[c], (c, s)
    return SA, SB, col_t, col_b, valid


# ----------------------------------------------------------- device kernel ---

def _build(SA, SB):
    nc = bacc.Bacc("TRN2", target_bir_lowering=False, debug=False)
    SIG = mybir.ActivationFunctionType.Sigmoid
    TANH = mybir.ActivationFunctionType.Tanh

    gi = {}   # per-group dram tensors / python state
    for g, S in (("A", SA), ("B", SB)):
        gi[g] = {
            "S": S,
            "xT": nc.dram_tensor(f"xT{g}", [S, 128, 4, SLOTS], F32R, kind="ExternalInput"),
            "m": nc.dram_tensor(f"m{g}", [128, S], F32, kind="ExternalInput"),
            "diag": nc.dram_tensor(f"diag{g}", [S - 1, 128, SLOTS], F32, kind="ExternalInput"),
            "hm0": nc.dram_tensor(f"hm0{g}", [128, 2, SLOTS], F32R, kind="ExternalInput"),
            "c0": nc.dram_tensor(f"c0{g}", [128, H], F32, kind="ExternalInput"),
            "hout": nc.dram_tensor(f"h_out{g}", [S, 128, H], F32, kind="ExternalOutput"),
            "cfin": nc.dram_tensor(f"c_fin{g}", [128, H], F32, kind="ExternalOutput"),
        }
    d_wih = nc.dram_tensor("wih", [128, 4, 1024], F32R, kind="ExternalInput")
    d_whh = nc.dram_tensor("whh", [128, 2, 1024], F32R, kind="ExternalInput")
    d_bias = nc.dram_tensor("bias", [1, 1024], F32R, kind="ExternalInput")
    d_ones = nc.dram_tensor("ones", [1, SLOTS], F32R, kind="ExternalInput")

    with tile.TileContext(nc) as tc:
        with (
            tc.tile_pool(name="const", bufs=1) as const,
            tc.tile_pool(name="xs", bufs=4) as xs,
            tc.tile_pool(name="acts", bufs=3) as acts,
            tc.tile_pool(name="state", bufs=2) as state,
            tc.tile_pool(name="hout", bufs=4) as houtp,
            tc.tile_pool(name="gps", bufs=3, space="PSUM") as gps,
            tc.tile_pool(name="tps", bufs=2, space="PSUM") as tps,
        ):
            wih = const.tile([128, 4, 1024], F32R)
            nc.sync.dma_start(wih[:], d_wih.ap()[:])
            whh = const.tile([128, 2, 1024], F32R)
            nc.sync.dma_start(whh[:], d_whh.ap()[:])
            bias = const.tile([1, 1024], F32R)
            nc.sync.dma_start(bias[:], d_bias.ap()[:])
            ones = const.tile([1, SLOTS], F32R)
            nc.sync.dma_start(ones[:], d_ones.ap()[:])
            for g in ("A", "B"):
                d = gi[g]
                d["msc"] = const.tile([128, d["S"]], F32, tag=f"msc{g}", name=f"msc{g}")
                nc.sync.dma_start(d["msc"][:], d["m"].ap()[:])
                d["diag_sb"] = const.tile([128, d["S"] - 1, SLOTS], F32, tag=f"diag{g}", name=f"diagsb{g}")
                nc.sync.dma_start(d["diag_sb"][:], d["diag"].ap()[:].rearrange("s p n -> p s n"))
                d["hmT"] = const.tile([128, 2, SLOTS], F32R, tag=f"hm0{g}", name=f"hmT0{g}")
                nc.sync.dma_start(d["hmT"][:], d["hm0"].ap()[:])
                d["c"] = const.tile([128, H], F32, tag=f"c0{g}", name=f"cc0{g}")
                nc.sync.dma_start(d["c"][:], d["c0"].ap()[:])
                d["gates"] = None

            def x_part(g, s):
                d = gi[g]
                xt = xs.tile([128, 4, SLOTS], F32R, tag="xt")
                nc.sync.dma_start(xt[:], d["xT"].ap()[s])
                gates = gps.tile([128, 1024], F32, tag="gates", name="gates")
                # k-major order: each lhsT is loaded once and used for both
                # N-halves back-to-back (halves LDWEIGHTS traffic)
                for k in range(4):
                    for nh in range(2):
                        n0 = nh * 512
                        nc.tensor.matmul(gates[:, n0:n0 + 512], xt[:, k, :],
                                         wih[:, k, n0:n0 + 512],
                                         start=(k == 0), stop=False)
                for nh in range(2):
                    n0 = nh * 512
                    nc.tensor.matmul(gates[:, n0:n0 + 512], ones[:],
                                     bias[:, n0:n0 + 512], start=False, stop=False)
                d["gates"] = gates

            def step_tail(g, s):
                """h-part matmuls + activations + state update for group g step s.
                Also prefetches this group's next x-part right after the h-part
                so PE has independent work while the chain tail runs."""
                d = gi[g]
                S = d["S"]
                gates = d["gates"]
                for j in range(2):
                    for nh in range(2):
                        n0 = nh * 512
                        nc.tensor.matmul(gates[:, n0:n0 + 512], d["hmT"][:, j, :],
                                         whh[:, j, n0:n0 + 512],
                                         start=False, stop=(j == 1))
                if s + 1 < S:
                    x_part(g, s + 1)

                # cm early: only needs previous c and the mask constant
                cm = acts.tile([128, H], F32, tag="cm")
                nc.vector.tensor_scalar_mul(cm[:], d["c"][:], d["msc"][:, s:s + 1])

                fio = acts.tile([128, 768], F32, tag="fio")
                nc.scalar.activation(fio[:], gates[:, 0:768], SIG)
                gg = acts.tile([128, 256], F32, tag="gg")
                nc.scalar.activation(gg[:], gates[:, 768:1024], TANH)

                fcm = acts.tile([128, H], F32, tag="fcm")
                nc.vector.tensor_mul(fcm[:], cm[:], fio[:, 0:256])
                ig = acts.tile([128, H], F32, tag="ig")
                nc.vector.tensor_mul(ig[:], fio[:, 256:512], gg[:])
                c_new = state.tile([128, H], F32, tag=f"c{g}")
                nc.vector.tensor_add(c_new[:], fcm[:], ig[:])

                tc_ = acts.tile([128, H], F32, tag="tc")
                nc.scalar.activation(tc_[:], c_new[:], TANH)
                h_t = houtp.tile([128, H], F32, tag="ht")
                nc.vector.tensor_mul(h_t[:], fio[:, 512:768], tc_[:])
                nc.sync.dma_start(d["hout"].ap()[s], h_t[:])

                if s < S - 1:
                    # hmT[h, slot] = h_t[slot, h] * m[slot, s+1]
                    mps = tps.tile([128, 2, SLOTS], F32, tag="mps")
                    for j in range(2):
                        nc.tensor.matmul(mps[:, j, :], h_t[:, j * 128:(j + 1) * 128],
                                         d["diag_sb"][:, s, :], start=True, stop=True)
                    hmT = state.tile([128, 2, SLOTS], F32R, tag=f"hmT{g}")
                    nc.vector.tensor_copy(hmT[:].rearrange("p j n -> p (j n)"),
                                          mps[:].rearrange("p j n -> p (j n)"))
                    d["hmT"] = hmT
                d["c"] = c_new

            # pipeline: interleave one B-step and one A-step per macro-step
            x_part("B", 0)
            x_part("A", 0)
            for s in range(SB):
                step_tail("B", s)
                if s < SA:
                    step_tail("A", s)

            nc.sync.dma_start(gi["A"]["cfin"].ap()[:], gi["A"]["c"][:])
            nc.sync.dma_start(gi["B"]["cfin"].ap()[:], gi["B"]["c"][:])

    nc.compile()
    return nc


_CACHE = {}


def _get_program(SA, SB):
    if (SA, SB) not in _CACHE:
        _CACHE[(SA, SB)] = _build(SA, SB)
    return _CACHE[(SA, SB)]


# ------------------------------------------------------------------ entry ---

def kernel(x, done, W_ih, W_hh, b_ih, b_hh, h0, c0):
    x = np.ascontiguousarray(np.asarray(x, np.float32))
    done = np.asarray(done).astype(np.int32)
    W_ih = np.asarray(W_ih, np.float32)
    W_hh = np.asarray(W_hh, np.float32)
    b_ih = np.asarray(b_ih, np.float32)
    b_hh = np.asarray(b_hh, np.float32)
    h0 = np.asarray(h0, np.float32)
    c0 = np.asarray(c0, np.float32)

    SA, SB, col_t, col_b, valid = _pack(done)
    Sg = {"A": SA, "B": SB}

    mask = (1 - done).astype(np.float32)
    m_col = mask[col_t, col_b] * valid            # [2*NCOLS, SB]

    wih_r = W_ih[_GATE_PERM]
    whh_r = W_hh[_GATE_PERM]
    bias_r = (b_ih + b_hh)[_GATE_PERM]
    wih_t = np.ascontiguousarray(wih_r.T.reshape(4, 128, 1024).transpose(1, 0, 2))
    whh_t = np.ascontiguousarray(whh_r.T.reshape(2, 128, 1024).transpose(1, 0, 2))
    bias_row = np.ascontiguousarray(bias_r[None, :])
    ones_row = np.ones((1, SLOTS), np.float32)

    # initial states: head columns (group A cols 0..63) carry h0/c0
    h_init = np.zeros((2 * NCOLS, H), np.float32)
    c_init = np.zeros((2 * NCOLS, H), np.float32)
    h_init[:B] = h0
    c_init[:B] = c0
    hm0 = h_init * m_col[:, 0:1]

    xp = x[col_t, col_b]                          # [2*NCOLS, SB, 512]

    in_maps = []
    for core in range(NCORES):
        im = {"wih": wih_t, "whh": whh_t, "bias": bias_row, "ones": ones_row}
        for gidx, g in enumerate(("A", "B")):
            S = Sg[g]
            base = gidx * NCOLS + core * SLOTS
            sl = slice(base, base + SLOTS)
            mc = m_col[sl, :S]                    # [SLOTS, S]
            dg = np.zeros((S - 1, 128, SLOTS), np.float32)
            idx = np.arange(SLOTS)
            for s in range(1, S):
                dg[s - 1, idx, idx] = mc[:, s]
            im[f"xT{g}"] = np.ascontiguousarray(
                xp[sl, :S].reshape(SLOTS, S, 4, 128).transpose(1, 3, 2, 0))
            im[f"m{g}"] = np.ascontiguousarray(mc)
            im[f"diag{g}"] = dg
            im[f"hm0{g}"] = np.ascontiguousarray(
                hm0[sl].reshape(SLOTS, 2, 128).transpose(2, 1, 0))
            im[f"c0{g}"] = np.ascontiguousarray(c_init[sl])
        in_maps.append(im)

    nc = _get_program(SA, SB)
    res = bass_utils.run_bass_kernel_spmd(nc, in_maps, core_ids=list(range(NCORES)))

    new_hidden = np.empty((T * B, H), np.float32)
    out_idx = col_t.astype(np.int64) * B + col_b
    for gidx, g in enumerate(("A", "B")):
        S = Sg[g]
        h_stack = np.stack([res.results[c][f"h_out{g}"] for c in range(NCORES)])
        h_cols = h_stack.transpose(0, 2, 1, 3).reshape(NCOLS, S, H)
        gsl = slice(gidx * NCOLS, (gidx + 1) * NCOLS)
        v = valid[gsl, :S]
        new_hidden[out_idx[gsl, :S][v]] = h_cols[v]

    hT = np.ascontiguousarray(new_hidden.reshape(T, B, H)[T - 1])
    c_finA0 = res.results[0]["c_finA"]            # tail cols: group A 64..127, core 0
    cT = np.ascontiguousarray(c_finA0[64:128])

    return new_hidden, hT[None], cT[None]


# revision 7
# speedup vs baseline: 1.0377x; 1.0377x over previous
"""Masked-LSTM scan (MemoryAgent) Trainium2 kernel.

Strategy: the done-mask (p=0.5 per step) zeroes the carried state, so each
batch row's 512-step timeline splits into independent episodes at reset
points. Episodes are bin-packed (host side, exact) into 2048 columns split
into two groups per core: group A columns of SA steps and group B columns of
SB steps (SA+SB=32), 128 columns of each group per core. The device kernel
interleaves one A-step and one B-step per macro-step — two independent
dependency chains that keep PE/ACT/DVE busy. Per step: gate matmuls (x-part
+ bias + recurrent part accumulated in PSUM, float32r), sigmoid/tanh on
ScalarE, c/h elementwise updates on VectorE, and the masked state transpose
for the next step fused into a TensorE matmul against a per-step diagonal
mask matrix.
"""

import sys

try:
    import concourse  # noqa: F401
except ImportError:  # pragma: no cover
    sys.path.insert(0, "/opt/trn_rl_repo")

import numpy as np

import concourse.bacc as bacc
import concourse.mybir as mybir
import concourse.tile as tile
from concourse import bass_utils

T, B, I, H = 512, 64, 512, 256
NCORES = 8
SLOTS = 128
NCOLS = NCORES * SLOTS          # columns per group
F32 = mybir.dt.float32
F32R = mybir.dt.float32r

# PyTorch gate order in the weights is [i, f, g, o]; we compute in order
# [f, i, o, g] so sigmoid covers one contiguous [0:768] slice and tanh [768:1024].
_GATE_PERM = np.concatenate([
    np.arange(256, 512),    # f
    np.arange(0, 256),      # i
    np.arange(768, 1024),   # o
    np.arange(512, 768),    # g
])


# ---------------------------------------------------------------- packing ---

def _episodes(done):
    heads, tails, mids = [], [], []
    for b in range(B):
        starts = np.flatnonzero(done[1:, b]) + 1
        starts = [0] + starts.tolist()
        ends = starts[1:] + [T]
        row = [(b, t0, e - t0) for t0, e in zip(starts, ends)]
        assert len(row) >= 2, "single-episode row unsupported"
        heads.append(row[0])
        tails.append(row[-1])
        mids.extend(row[1:-1])
    return heads, tails, mids


def _pack(done):
    """Bin-pack episodes into two groups of NCOLS columns: group A with SA
    steps, group B with SB steps, SA+SB = T*B/NCOLS.

    Heads (carrying h0/c0) start A-columns 0..63; tails (whose final c is
    cT[b]) end A-columns 64..127. Episodes longer than SA go to B columns.
    Returns (SA, SB, col_t, col_b, valid) with col_* of shape [2*NCOLS, Smax]
    (group A rows first), padded positions invalid.
    """
    heads, tails, mids = _episodes(done)
    stot = (T * B) // NCOLS                      # SA + SB
    maxmid = max(l for (_, _, l) in mids)
    maxht = max(l for (_, _, l) in heads + tails)
    SB = max(maxmid, (stot + 1) // 2)
    SA = stot - SB
    assert SA >= 1 and maxht <= SA, (SA, SB, maxht)

    nb = 2 * NCOLS
    cap = np.array([SA] * NCOLS + [SB] * NCOLS)
    head_of = [None] * nb
    tail_of = [None] * nb
    mids_of = [[] for _ in range(nb)]
    rem = cap.copy()
    for b, ep in enumerate(heads):
        head_of[b] = ep
        rem[b] -= ep[2]
    for b, ep in enumerate(tails):
        tail_of[64 + b] = ep
        rem[64 + b] -= ep[2]
    assert (rem >= 0).all()

    big = sorted([m for m in mids if m[2] > 1], key=lambda e: -e[2])
    unit = [m for m in mids if m[2] == 1]

    by_rem = [[] for _ in range(SB + 1)]
    for c in range(nb):
        by_rem[rem[c]].append(c)
    for ep in big:
        L = ep[2]
        for r in range(L, SB + 1):
            if by_rem[r]:
                c = by_rem[r].pop()
                mids_of[c].append(ep)
                rem[c] -= L
                by_rem[r - L].append(c)
                break
        else:
            raise AssertionError("packing failed for episode of length %d" % L)
    ui = 0
    for c in range(nb):
        while rem[c] > 0 and ui < len(unit):
            mids_of[c].append(unit[ui])
            ui += 1
            rem[c] -= 1
    assert ui == len(unit) and (rem == 0).all()

    Smax = SB
    col_t = np.zeros((nb, Smax), np.int32)
    col_b = np.zeros((nb, Smax), np.int32)
    valid = np.zeros((nb, Smax), bool)
    for c in range(nb):
        items = []
        if head_of[c]:
            items.append(head_of[c])
        items.extend(mids_of[c])
        if tail_of[c]:
            items.append(tail_of[c])
        s = 0
        for b, t0, L in items:
            col_t[c, s:s + L] = np.arange(t0, t0 + L)
            col_b[c, s:s + L] = b
            valid[c, s:s + L] = True
            s += L
        assert s == cap[c], (c, s)
    return SA, SB, col_t, col_b, valid


# ----------------------------------------------------------- device kernel ---

def _build(SA, SB):
    nc = bacc.Bacc("TRN2", target_bir_lowering=False, debug=False)
    SIG = mybir.ActivationFunctionType.Sigmoid
    TANH = mybir.ActivationFunctionType.Tanh

    gi = {}   # per-group dram tensors / python state
    for g, S in (("A", SA), ("B", SB)):
        gi[g] = {
            "S": S,
            "xT": nc.dram_tensor(f"xT{g}", [S, 128, 4, SLOTS], F32R, kind="ExternalInput"),
            "m": nc.dram_tensor(f"m{g}", [128, S], F32, kind="ExternalInput"),
            "diag": nc.dram_tensor(f"diag{g}", [S - 1, 128, SLOTS], F32, kind="ExternalInput"),
            "hm0": nc.dram_tensor(f"hm0{g}", [128, 2, SLOTS], F32R, kind="ExternalInput"),
            "c0": nc.dram_tensor(f"c0{g}", [128, H], F32, kind="ExternalInput"),
            "hout": nc.dram_tensor(f"h_out{g}", [S, 128, H], F32, kind="ExternalOutput"),
            "cfin": nc.dram_tensor(f"c_fin{g}", [128, H], F32, kind="ExternalOutput"),
        }
    d_wih = nc.dram_tensor("wih", [128, 4, 1024], F32R, kind="ExternalInput")
    d_whh = nc.dram_tensor("whh", [128, 2, 1024], F32R, kind="ExternalInput")
    d_bias = nc.dram_tensor("bias", [1, 1024], F32R, kind="ExternalInput")
    d_ones = nc.dram_tensor("ones", [1, SLOTS], F32R, kind="ExternalInput")

    with tile.TileContext(nc) as tc:
        with (
            tc.tile_pool(name="const", bufs=1) as const,
            tc.tile_pool(name="xs", bufs=4) as xs,
            tc.tile_pool(name="acts", bufs=3) as acts,
            tc.tile_pool(name="state", bufs=2) as state,
            tc.tile_pool(name="hout", bufs=4) as houtp,
            tc.tile_pool(name="gps", bufs=3, space="PSUM") as gps,
            tc.tile_pool(name="tps", bufs=2, space="PSUM") as tps,
        ):
            wih = const.tile([128, 4, 1024], F32R)
            nc.sync.dma_start(wih[:], d_wih.ap()[:])
            whh = const.tile([128, 2, 1024], F32R)
            nc.sync.dma_start(whh[:], d_whh.ap()[:])
            bias = const.tile([1, 1024], F32R)
            nc.sync.dma_start(bias[:], d_bias.ap()[:])
            ones = const.tile([1, SLOTS], F32R)
            nc.sync.dma_start(ones[:], d_ones.ap()[:])
            for g in ("A", "B"):
                d = gi[g]
                d["msc"] = const.tile([128, d["S"]], F32, tag=f"msc{g}", name=f"msc{g}")
                nc.sync.dma_start(d["msc"][:], d["m"].ap()[:])
                d["diag_sb"] = const.tile([128, d["S"] - 1, SLOTS], F32, tag=f"diag{g}", name=f"diagsb{g}")
                nc.sync.dma_start(d["diag_sb"][:], d["diag"].ap()[:].rearrange("s p n -> p s n"))
                d["hmT"] = const.tile([128, 2, SLOTS], F32R, tag=f"hm0{g}", name=f"hmT0{g}")
                nc.sync.dma_start(d["hmT"][:], d["hm0"].ap()[:])
                d["c"] = const.tile([128, H], F32, tag=f"c0{g}", name=f"cc0{g}")
                nc.sync.dma_start(d["c"][:], d["c0"].ap()[:])
                d["gates"] = None

            def x_part(g, s):
                d = gi[g]
                xt = xs.tile([128, 4, SLOTS], F32R, tag="xt")
                nc.sync.dma_start(xt[:], d["xT"].ap()[s])
                gates = gps.tile([128, 1024], F32, tag="gates", name="gates")
                # k-major order: each lhsT is loaded once and used for both
                # N-halves back-to-back (halves LDWEIGHTS traffic)
                for k in range(4):
                    for nh in range(2):
                        n0 = nh * 512
                        nc.tensor.matmul(gates[:, n0:n0 + 512], xt[:, k, :],
                                         wih[:, k, n0:n0 + 512],
                                         start=(k == 0), stop=False)
                for nh in range(2):
                    n0 = nh * 512
                    nc.tensor.matmul(gates[:, n0:n0 + 512], ones[:],
                                     bias[:, n0:n0 + 512], start=False, stop=False)
                d["gates"] = gates

            def step_tail(g, s):
                """h-part matmuls + activations + state update for group g step s.
                Also prefetches this group's next x-part right after the h-part
                so PE has independent work while the chain tail runs."""
                d = gi[g]
                S = d["S"]
                gates = d["gates"]
                for j in range(2):
                    for nh in range(2):
                        n0 = nh * 512
                        nc.tensor.matmul(gates[:, n0:n0 + 512], d["hmT"][:, j, :],
                                         whh[:, j, n0:n0 + 512],
                                         start=False, stop=(j == 1))
                if s + 1 < S:
                    x_part(g, s + 1)

                # cm early: only needs previous c and the mask constant
                cm = acts.tile([128, H], F32, tag="cm")
                nc.vector.tensor_scalar_mul(cm[:], d["c"][:], d["msc"][:, s:s + 1])

                fio = acts.tile([128, 768], F32, tag="fio")
                nc.scalar.activation(fio[:], gates[:, 0:768], SIG)
                gg = acts.tile([128, 256], F32, tag="gg")
                nc.scalar.activation(gg[:], gates[:, 768:1024], TANH)

                fcm = acts.tile([128, H], F32, tag="fcm")
                nc.vector.tensor_mul(fcm[:], cm[:], fio[:, 0:256])
                ig = acts.tile([128, H], F32, tag="ig")
                nc.vector.tensor_mul(ig[:], fio[:, 256:512], gg[:])
                c_new = state.tile([128, H], F32, tag=f"c{g}")
                nc.vector.tensor_add(c_new[:], fcm[:], ig[:])

                tc_ = acts.tile([128, H], F32, tag="tc")
                nc.scalar.activation(tc_[:], c_new[:], TANH)
                h_t = houtp.tile([128, H], F32, tag="ht")
                nc.vector.tensor_mul(h_t[:], fio[:, 512:768], tc_[:])
                nc.sync.dma_start(d["hout"].ap()[s], h_t[:])

                if s < S - 1:
                    # hmT[h, slot] = h_t[slot, h] * m[slot, s+1]
                    mps = tps.tile([128, 2, SLOTS], F32, tag="mps")
                    for j in range(2):
                        # transpose-mode matmul (2 cyc/row fp32 vs 4): same
                        # lhsT.T @ rhs math, rhs = per-step diagonal mask
                        nc.tensor.transpose(mps[:, j, :], h_t[:, j * 128:(j + 1) * 128],
                                            d["diag_sb"][:, s, :])
                    hmT = state.tile([128, 2, SLOTS], F32R, tag=f"hmT{g}")
                    nc.vector.tensor_copy(hmT[:].rearrange("p j n -> p (j n)"),
                                          mps[:].rearrange("p j n -> p (j n)"))
                    d["hmT"] = hmT
                d["c"] = c_new

            # pipeline: interleave one B-step and one A-step per macro-step
            x_part("B", 0)
            x_part("A", 0)
            for s in range(SB):
                step_tail("B", s)
                if s < SA:
                    step_tail("A", s)

            nc.sync.dma_start(gi["A"]["cfin"].ap()[:], gi["A"]["c"][:])
            nc.sync.dma_start(gi["B"]["cfin"].ap()[:], gi["B"]["c"][:])

    nc.compile()
    return nc


_CACHE = {}


def _get_program(SA, SB):
    if (SA, SB) not in _CACHE:
        _CACHE[(SA, SB)] = _build(SA, SB)
    return _CACHE[(SA, SB)]


# ------------------------------------------------------------------ entry ---

def kernel(x, done, W_ih, W_hh, b_ih, b_hh, h0, c0):
    x = np.ascontiguousarray(np.asarray(x, np.float32))
    done = np.asarray(done).astype(np.int32)
    W_ih = np.asarray(W_ih, np.float32)
    W_hh = np.asarray(W_hh, np.float32)
    b_ih = np.asarray(b_ih, np.float32)
    b_hh = np.asarray(b_hh, np.float32)
    h0 = np.asarray(h0, np.float32)
    c0 = np.asarray(c0, np.float32)

    SA, SB, col_t, col_b, valid = _pack(done)
    Sg = {"A": SA, "B": SB}

    mask = (1 - done).astype(np.float32)
    m_col = mask[col_t, col_b] * valid            # [2*NCOLS, SB]

    wih_r = W_ih[_GATE_PERM]
    whh_r = W_hh[_GATE_PERM]
    bias_r = (b_ih + b_hh)[_GATE_PERM]
    wih_t = np.ascontiguousarray(wih_r.T.reshape(4, 128, 1024).transpose(1, 0, 2))
    whh_t = np.ascontiguousarray(whh_r.T.reshape(2, 128, 1024).transpose(1, 0, 2))
    bias_row = np.ascontiguousarray(bias_r[None, :])
    ones_row = np.ones((1, SLOTS), np.float32)

    # initial states: head columns (group A cols 0..63) carry h0/c0
    h_init = np.zeros((2 * NCOLS, H), np.float32)
    c_init = np.zeros((2 * NCOLS, H), np.float32)
    h_init[:B] = h0
    c_init[:B] = c0
    hm0 = h_init * m_col[:, 0:1]

    xp = x[col_t, col_b]                          # [2*NCOLS, SB, 512]

    in_maps = []
    for core in range(NCORES):
        im = {"wih": wih_t, "whh": whh_t, "bias": bias_row, "ones": ones_row}
        for gidx, g in enumerate(("A", "B")):
            S = Sg[g]
            base = gidx * NCOLS + core * SLOTS
            sl = slice(base, base + SLOTS)
            mc = m_col[sl, :S]                    # [SLOTS, S]
            dg = np.zeros((S - 1, 128, SLOTS), np.float32)
            idx = np.arange(SLOTS)
            for s in range(1, S):
                dg[s - 1, idx, idx] = mc[:, s]
            im[f"xT{g}"] = np.ascontiguousarray(
                xp[sl, :S].reshape(SLOTS, S, 4, 128).transpose(1, 3, 2, 0))
            im[f"m{g}"] = np.ascontiguousarray(mc)
            im[f"diag{g}"] = dg
            im[f"hm0{g}"] = np.ascontiguousarray(
                hm0[sl].reshape(SLOTS, 2, 128).transpose(2, 1, 0))
            im[f"c0{g}"] = np.ascontiguousarray(c_init[sl])
        in_maps.append(im)

    nc = _get_program(SA, SB)
    res = bass_utils.run_bass_kernel_spmd(nc, in_maps, core_ids=list(range(NCORES)))

    new_hidden = np.empty((T * B, H), np.float32)
    out_idx = col_t.astype(np.int64) * B + col_b
    for gidx, g in enumerate(("A", "B")):
        S = Sg[g]
        h_stack = np.stack([res.results[c][f"h_out{g}"] for c in range(NCORES)])
        h_cols = h_stack.transpose(0, 2, 1, 3).reshape(NCOLS, S, H)
        gsl = slice(gidx * NCOLS, (gidx + 1) * NCOLS)
        v = valid[gsl, :S]
        new_hidden[out_idx[gsl, :S][v]] = h_cols[v]

    hT = np.ascontiguousarray(new_hidden.reshape(T, B, H)[T - 1])
    c_finA0 = res.results[0]["c_finA"]            # tail cols: group A 64..127, core 0
    cT = np.ascontiguousarray(c_finA0[64:128])

    return new_hidden, hT[None], cT[None]
